# revision 1
# baseline (speedup 1.0000x reference)
"""Trainium2 Bass kernel for nn_Attention_46901042872408.

Dense MHA transformer block with RoPE + prefix-tuning branch:
  q/k/v = x @ wq/wk/wv; rope(q), rope(k); causal attention;
  prefix branch: non-causal attention of q against (prefix @ wk/wv),
  gated by tanh(prefix_gate) per head; out = (attn + gate*prefix_attn) @ wo.

Sharding: 8 cores = data-parallel over batch (2) x tensor-parallel over
heads (4 groups of 8 heads). Each core computes a partial [2048, 4096]
output (its heads' contribution through its wo row-slice); host sums the
4 partials per batch.

Per-core pipeline (all matmuls in float32r, 1 cycle/row on the PE):
  Phase 1: qkv projections in straight [token, col] layout (tokens on
    partitions) streaming weight column-blocks of 256 against a cached
    x^T token-chunk; RoPE applied on adjacent even/odd column pairs via
    strided APs; q/k transposed to [hd, token] via PE-transpose and
    spilled to DRAM; v spilled straight. Prefix k/v projections reuse the
    same streamed weight tiles against a resident prefix^T.
  Phase 2: per (head, 512-token q-block): scores^T tiles [k_tok, q_tok]
    via one matmul per 128-k-tile, exp on ACT, causal mask multiply on
    diagonal tiles (skip fully-masked tiles entirely), PV and ones-vector
    denominator matmuls accumulated in PSUM, prefix branch the same with
    30 k-rows, then combine with reciprocal + gpsimd partition-broadcast.
  Phase 3: out_partial = attnT.T @ wo with wo fully cached in SBUF.
"""

import sys

sys.path.insert(0, "/opt/trn_rl_repo")

import numpy as np

B, S, D = 2, 2048, 4096
H, HD = 32, 128
PFX = 30
NCORES = 8
CPB = 4  # cores per batch (head-parallel groups)
HPC = 8  # heads per core
COLS = HPC * HD  # 1024 qkv columns per core
WB_COLS = 256  # weight column-block
NKT = D // 128  # 32 contraction tiles
import os as _os

CHUNKS = (
    [(0, 1024), (1024, 1024)]
    if _os.environ.get("CH") == "1024"
    else [(0, 896), (896, 896), (1792, 256)]
)
SCALE = 1.0 / float(np.sqrt(HD))

_CACHE = {}


def _build(mm_fp32r=True):
    import os
    from contextlib import ExitStack

    phases = os.environ.get("KPHASES", "123")

    def knob(name, default):
        return int(os.environ.get(name, default))

    import concourse.tile as tile
    from concourse import bacc, mybir

    f32 = mybir.dt.float32
    mdt = mybir.dt.float32r if mm_fp32r else mybir.dt.float32
    AF = mybir.ActivationFunctionType
    OP = mybir.AluOpType

    nc = bacc.Bacc("TRN2", target_bir_lowering=False, debug=False, num_devices=NCORES)

    xT = nc.dram_tensor("xT", [D, S], mdt, kind="ExternalInput")
    wqkv = nc.dram_tensor("wqkv", [D, 3 * COLS], mdt, kind="ExternalInput")
    wo_d = nc.dram_tensor("wo", [COLS, D], mdt, kind="ExternalInput")
    pfT = nc.dram_tensor("pfT", [D, PFX], mdt, kind="ExternalInput")
    cosS = nc.dram_tensor("cosS", [S, 128], f32, kind="ExternalInput")
    sinS = nc.dram_tensor("sinS", [S, 128], f32, kind="ExternalInput")
    masks = nc.dram_tensor("masks", [128, 4, 512], f32, kind="ExternalInput")
    ones_d = nc.dram_tensor("ones", [128, 1], mdt, kind="ExternalInput")
    eye_d = nc.dram_tensor("eye", [128, 128], mdt, kind="ExternalInput")
    g_d = nc.dram_tensor("g", [1, HPC], f32, kind="ExternalInput")
    out_d = nc.dram_tensor("out", [S, D], f32, kind="ExternalOutput")

    with tile.TileContext(nc) as tc:
        with ExitStack() as top:
            dram = top.enter_context(tc.tile_pool(name="dram", bufs=1, space="DRAM"))
            qkT_sp = dram.tile([2 * COLS, S], mdt)  # q rows 0..1023, k rows 1024..2047
            v_sp = dram.tile([S, COLS], mdt)
            att_sp = dram.tile([COLS, S], mdt)

            pres = top.enter_context(tc.tile_pool(name="res", bufs=1))
            pf_resident = os.environ.get("PFRES", "1") == "1"
            if pf_resident:
                pf_sb = pres.tile([128, NKT, PFX], mdt)
                nc.sync.dma_start(
                    pf_sb[:], pfT[:].rearrange("(ko p) n -> p ko n", p=128)
                )
            eye_sb = pres.tile([128, 128], mdt)
            nc.sync.dma_start(eye_sb[:], eye_d[:])
            ones_sb = pres.tile([128, 1], mdt)
            nc.sync.dma_start(ones_sb[:], ones_d[:])
            g_sb = pres.tile([1, HPC], f32)
            nc.sync.dma_start(g_sb[:], g_d[:])
            pkT_sb = pres.tile([128, HPC, PFX], mdt)
            pv_sb = pres.tile([PFX, 4, 2 * 128], mdt)  # straight prefix-v, 2 heads/block

            # ---------------- Phase 1: projections ----------------
            with ExitStack() as ph1:
              if "1" in phases:
                px = ph1.enter_context(tc.tile_pool(name="px", bufs=1))
                pw = ph1.enter_context(tc.tile_pool(name="pw", bufs=knob("B_pw", 2)))
                pcs = ph1.enter_context(tc.tile_pool(name="pcs", bufs=knob("B_pcs", 3)))
                ptmp = ph1.enter_context(tc.tile_pool(name="ptmp", bufs=knob("B_ptmp", 2)))
                po = ph1.enter_context(tc.tile_pool(name="po", bufs=knob("B_po", 3)))
                poT = ph1.enter_context(tc.tile_pool(name="poT", bufs=knob("B_poT", 3)))
                ppk = ph1.enter_context(tc.tile_pool(name="ppk", bufs=knob("B_ppk", 2)))
                ps_mm = ph1.enter_context(
                    tc.tile_pool(name="ps_mm", bufs=knob("B_psmm", 4), space="PSUM")
                )
                ps_tr = ph1.enter_context(
                    tc.tile_pool(name="ps_tr", bufs=knob("B_pstr", 2), space="PSUM")
                )
                ps_pk = ph1.enter_context(
                    tc.tile_pool(name="ps_pk", bufs=1, space="PSUM")
                )
                ps_ptr = ph1.enter_context(
                    tc.tile_pool(name="ps_ptr", bufs=1, space="PSUM")
                )

                for ck, (tb, ntok) in enumerate(CHUNKS):
                    nmt = ntok // 128
                    x0 = px.tile([128, NKT // 2, ntok], mdt, tag="x0")
                    x1 = px.tile([128, NKT // 2, ntok], mdt, tag="x1")
                    nc.sync.dma_start(
                        x0[:],
                        xT[0 : D // 2, tb : tb + ntok].rearrange(
                            "(ko p) n -> p ko n", p=128
                        ),
                    )
                    nc.sync.dma_start(
                        x1[:],
                        xT[D // 2 : D, tb : tb + ntok].rearrange(
                            "(ko p) n -> p ko n", p=128
                        ),
                    )
                    for wb in range(12):
                        w_sb = pw.tile([128, NKT, WB_COLS], mdt, tag="w")
                        nc.sync.dma_start(
                            w_sb[:],
                            wqkv[:, wb * WB_COLS : (wb + 1) * WB_COLS].rearrange(
                                "(ko p) c -> p ko c", p=128
                            ),
                        )
                        if ck == 0 and wb >= 4:
                            # prefix projections off the same weight stream
                            psp = ps_pk.tile([PFX, WB_COLS], f32, tag="ppk")
                            for ki in range(NKT):
                                if pf_resident:
                                    pf_t = pf_sb[:, ki, :]
                                else:
                                    pf_tile = ppk.tile([128, PFX], mdt, tag="pf")
                                    nc.sync.dma_start(
                                        pf_tile[:],
                                        pfT[ki * 128 : (ki + 1) * 128, :],
                                    )
                                    pf_t = pf_tile[:]
                                nc.tensor.matmul(
                                    psp[:],
                                    lhsT=pf_t,
                                    rhs=w_sb[:, ki, :],
                                    start=(ki == 0),
                                    stop=(ki == NKT - 1),
                                )
                            if wb < 8:  # k-cols -> pkT (transposed per head)
                                pks = ppk.tile([PFX, WB_COLS], mdt, tag="pks")
                                nc.scalar.activation(pks[:], psp[:], AF.Copy)
                                for c in range(2):
                                    h = (wb - 4) * 2 + c
                                    ptr = ps_ptr.tile([128, PFX], mdt, tag="ptr")
                                    nc.tensor.transpose(
                                        ptr[:],
                                        pks[:, c * 128 : (c + 1) * 128],
                                        eye_sb[0:PFX, 0:PFX],
                                    )
                                    nc.vector.tensor_copy(
                                        pkT_sb[:, h, :], ptr[:].bitcast(f32)
                                    )
                            else:  # v-cols -> straight prefix-v
                                nc.scalar.activation(
                                    pv_sb[:, wb - 8, :], psp[:], AF.Copy
                                )
                        for mt in range(nmt):
                            ps = ps_mm.tile([128, WB_COLS], f32, tag="mm")
                            for ki in range(NKT):
                                xs = x0 if ki < NKT // 2 else x1
                                nc.tensor.matmul(
                                    ps[:],
                                    lhsT=xs[:, ki % (NKT // 2), mt * 128 : (mt + 1) * 128],
                                    rhs=w_sb[:, ki, :],
                                    start=(ki == 0),
                                    stop=(ki == NKT - 1),
                                )
                            tok0 = tb + mt * 128
                            if wb < 8:  # q/k: rope, transpose, spill
                                cc = pcs.tile([128, 128], f32, tag="cos")
                                ss = pcs.tile([128, 128], f32, tag="sin")
                                nc.sync.dma_start(
                                    cc[:], cosS[tok0 : tok0 + 128, :]
                                )
                                nc.sync.dma_start(
                                    ss[:], sinS[tok0 : tok0 + 128, :]
                                )
                                p3 = ps[:].rearrange("p (i two) -> p i two", two=2)
                                o = po.tile([128, WB_COLS], mdt, tag="o")
                                o3 = o[:].rearrange("p (i two) -> p i two", two=2)
                                m1 = ptmp.tile([128, 128], f32, tag="m1")
                                m2 = ptmp.tile([128, 128], f32, tag="m2")
                                nc.vector.tensor_tensor(
                                    m1[:], p3[:, :, 0], cc[:], OP.mult
                                )
                                nc.vector.tensor_tensor(
                                    m2[:], p3[:, :, 1], ss[:], OP.mult
                                )
                                nc.vector.tensor_tensor(
                                    o3[:, :, 0], m1[:], m2[:], OP.subtract
                                )
                                m3 = ptmp.tile([128, 128], f32, tag="m1")
                                m4 = ptmp.tile([128, 128], f32, tag="m2")
                                nc.vector.tensor_tensor(
                                    m3[:], p3[:, :, 0], ss[:], OP.mult
                                )
                                nc.vector.tensor_tensor(
                                    m4[:], p3[:, :, 1], cc[:], OP.mult
                                )
                                nc.vector.tensor_tensor(
                                    o3[:, :, 1], m3[:], m4[:], OP.add
                                )
                                for c in range(2):
                                    ptr2 = ps_tr.tile([128, 128], mdt, tag="tr")
                                    nc.tensor.transpose(
                                        ptr2[:],
                                        o[:, c * 128 : (c + 1) * 128],
                                        eye_sb[:],
                                    )
                                    oT = poT.tile([128, 128], mdt, tag="oT")
                                    nc.scalar.activation(
                                        oT[:], ptr2[:].bitcast(f32), AF.Copy
                                    )
                                    row0 = wb * WB_COLS + c * 128
                                    nc.sync.dma_start(
                                        qkT_sp[row0 : row0 + 128, tok0 : tok0 + 128],
                                        oT[:],
                                    )
                            else:  # v: copy out straight
                                o = po.tile([128, WB_COLS], mdt, tag="o")
                                nc.scalar.activation(o[:], ps[:], AF.Copy)
                                col0 = (wb - 8) * WB_COLS
                                nc.sync.dma_start(
                                    v_sp[tok0 : tok0 + 128, col0 : col0 + WB_COLS],
                                    o[:],
                                )

            if not os.environ.get("NOBAR12"):
                tc.strict_bb_all_engine_barrier()
            # ---------------- Phase 2: attention ----------------
            with ExitStack() as ph2:
              if True:
                pwo = ph2.enter_context(tc.tile_pool(name="pwo", bufs=1))
                with ExitStack() as ph2i:
                  if "2" in phases:
                    wo_h0 = pwo.tile([128, 4, D], mdt, tag="wo0")
                    nc.sync.dma_start(
                        wo_h0[:], wo_d[0 : COLS // 2, :].rearrange("(ko p) d -> p ko d", p=128)
                    )
                    pmask = ph2i.enter_context(tc.tile_pool(name="pmask", bufs=1))
                    masks_sb = pmask.tile([128, 4, 512], f32)
                    nc.sync.dma_start(masks_sb[:], masks[:])
                    pkv = ph2i.enter_context(tc.tile_pool(name="pkv", bufs=knob("B_pkv", 2)))
                    pq = ph2i.enter_context(tc.tile_pool(name="pq", bufs=2))
                    pE = ph2i.enter_context(tc.tile_pool(name="pE", bufs=knob("B_pE", 4)))
                    pc = ph2i.enter_context(tc.tile_pool(name="pc", bufs=2))
                    ps_s = ph2i.enter_context(
                        tc.tile_pool(name="ps_s", bufs=knob("B_pss", 2), space="PSUM")
                    )
                    ps_pv = ph2i.enter_context(
                        tc.tile_pool(name="ps_pv", bufs=2, space="PSUM")
                    )
                    ps_den = ph2i.enter_context(
                        tc.tile_pool(name="ps_den", bufs=1, space="PSUM")
                    )
                    ps_pfx = ph2i.enter_context(
                        tc.tile_pool(name="ps_pfx", bufs=1, space="PSUM")
                    )

                    for h in range(HPC):
                        kT = pkv.tile([128, S], mdt, tag="kT")
                        nc.sync.dma_start(
                            kT[:], qkT_sp[COLS + h * 128 : COLS + (h + 1) * 128, :]
                        )
                        vv = pkv.tile([128, S // 128, 128], mdt, tag="v")
                        nc.sync.dma_start(
                            vv[:],
                            v_sp[:, h * 128 : (h + 1) * 128].rearrange(
                                "(kb p) c -> p kb c", p=128
                            ),
                        )
                        for qb in range(4):
                            q_sb = pq.tile([128, 512], mdt, tag="q")
                            nc.sync.dma_start(
                                q_sb[:],
                                qkT_sp[h * 128 : (h + 1) * 128, qb * 512 : (qb + 1) * 512],
                            )
                            nkb = 4 * qb + 4
                            pv_ps = ps_pv.tile([128, 512], f32, tag="pv")
                            den_ps = ps_den.tile([1, 512], f32, tag="den")
                            for kb in range(nkb):
                                s_ps = ps_s.tile([128, 512], f32, tag="s")
                                nc.tensor.matmul(
                                    s_ps[:],
                                    lhsT=kT[:, kb * 128 : (kb + 1) * 128],
                                    rhs=q_sb[:],
                                    start=True,
                                    stop=True,
                                )
                                E = pE.tile([128, 512], mdt, tag="E")
                                nc.scalar.activation(
                                    E[:], s_ps[:], AF.Exp, scale=SCALE
                                )
                                t = kb - 4 * qb
                                if t >= 0:
                                    nc.vector.tensor_tensor(
                                        E[:],
                                        E[:].bitcast(f32),
                                        masks_sb[:, t, :],
                                        OP.mult,
                                    )
                                nc.tensor.matmul(
                                    pv_ps[:],
                                    lhsT=vv[:, kb, :],
                                    rhs=E[:],
                                    start=(kb == 0),
                                    stop=(kb == nkb - 1),
                                )
                                nc.tensor.matmul(
                                    den_ps[:],
                                    lhsT=ones_sb[:],
                                    rhs=E[:],
                                    start=(kb == 0),
                                    stop=(kb == nkb - 1),
                                )
                            # prefix branch
                            sp_ps = ps_pfx.tile([PFX, 512], f32, tag="sp")
                            nc.tensor.matmul(
                                sp_ps[:],
                                lhsT=pkT_sb[:, h, :],
                                rhs=q_sb[:],
                                start=True,
                                stop=True,
                            )
                            EP = pE.tile([PFX, 512], mdt, tag="EP")
                            nc.scalar.activation(EP[:], sp_ps[:], AF.Exp, scale=SCALE)
                            pvP_ps = ps_pfx.tile([128, 512], f32, tag="pvP")
                            nc.tensor.matmul(
                                pvP_ps[:],
                                lhsT=pv_sb[:, h // 2, (h % 2) * 128 : (h % 2) * 128 + 128],
                                rhs=EP[:],
                                start=True,
                                stop=True,
                            )
                            denP_ps = ps_pfx.tile([1, 512], f32, tag="denP")
                            nc.tensor.matmul(
                                denP_ps[:],
                                lhsT=ones_sb[0:PFX, :],
                                rhs=EP[:],
                                start=True,
                                stop=True,
                            )
                            # combine: att = pv/den + g * pvP/denP
                            r1 = pc.tile([1, 512], f32, tag="r1")
                            nc.vector.reciprocal(r1[:], den_ps[:])
                            r2 = pc.tile([1, 512], f32, tag="r2")
                            nc.vector.reciprocal(r2[:], denP_ps[:])
                            nc.vector.tensor_scalar_mul(
                                r2[:], r2[:], g_sb[0:1, h : h + 1]
                            )
                            rb1 = pc.tile([128, 512], f32, tag="rb1")
                            nc.gpsimd.partition_broadcast(rb1[:], r1[:])
                            rb2 = pc.tile([128, 512], f32, tag="rb2")
                            nc.gpsimd.partition_broadcast(rb2[:], r2[:])
                            t1 = pc.tile([128, 512], f32, tag="t1")
                            nc.vector.tensor_tensor(t1[:], pv_ps[:], rb1[:], OP.mult)
                            t2 = pc.tile([128, 512], f32, tag="t2")
                            nc.vector.tensor_tensor(t2[:], pvP_ps[:], rb2[:], OP.mult)
                            att = pc.tile([128, 512], mdt, tag="att")
                            nc.vector.tensor_tensor(att[:], t1[:], t2[:], OP.add)
                            nc.sync.dma_start(
                                att_sp[h * 128 : (h + 1) * 128, qb * 512 : (qb + 1) * 512],
                                att[:],
                            )

                if not os.environ.get("NOBAR23"):
                    tc.strict_bb_all_engine_barrier()
                # ---------------- Phase 3: output projection ----------------
                with ExitStack() as ph3:
                  if "3" in phases:
                    pwo1 = ph3.enter_context(tc.tile_pool(name="pwo1", bufs=1))
                    wo_h1 = pwo1.tile([128, 4, D], mdt, tag="wo1")
                    nc.sync.dma_start(
                        wo_h1[:], wo_d[COLS // 2 : COLS, :].rearrange("(ko p) d -> p ko d", p=128)
                    )
                    pa = ph3.enter_context(tc.tile_pool(name="pa", bufs=2))
                    pout = ph3.enter_context(tc.tile_pool(name="pout", bufs=3))
                    ps3 = ph3.enter_context(
                        tc.tile_pool(name="ps3", bufs=knob("B_ps3", 4), space="PSUM")
                    )
                    for mt in range(S // 128):
                        a_sb = pa.tile([128, COLS // 128, 128], mdt, tag="a")
                        nc.sync.dma_start(
                            a_sb[:],
                            att_sp[:, mt * 128 : (mt + 1) * 128].rearrange(
                                "(kc p) t -> p kc t", p=128
                            ),
                        )
                        for nb in range(D // 512):
                            ps = ps3.tile([128, 512], f32, tag="mm3")
                            for kc in range(COLS // 128):
                                wo_half = wo_h0 if kc < 4 else wo_h1
                                nc.tensor.matmul(
                                    ps[:],
                                    lhsT=a_sb[:, kc % 4, :] if False else a_sb[:, kc, :],
                                    rhs=wo_half[:, kc % 4, nb * 512 : (nb + 1) * 512],
                                    start=(kc == 0),
                                    stop=(kc == COLS // 128 - 1),
                                )
                            o = pout.tile([128, 512], f32, tag="o3")
                            nc.scalar.activation(o[:], ps[:], AF.Copy)
                            nc.sync.dma_start(
                                out_d[mt * 128 : (mt + 1) * 128, nb * 512 : (nb + 1) * 512],
                                o[:],
                            )

    nc.compile()
    return nc


def _host_inputs(x, freqs_cos, freqs_sin, prefix, prefix_gate, wq, wk, wv, wo):
    x = np.asarray(x, np.float32)
    freqs_cos = np.asarray(freqs_cos, np.float32)
    freqs_sin = np.asarray(freqs_sin, np.float32)
    prefix = np.asarray(prefix, np.float32)
    prefix_gate = np.asarray(prefix_gate, np.float32)
    wq = np.asarray(wq, np.float32)
    wk = np.asarray(wk, np.float32)
    wv = np.asarray(wv, np.float32)
    wo = np.asarray(wo, np.float32)

    cosS = np.ascontiguousarray(np.tile(freqs_cos, (1, 2)))
    sinS = np.ascontiguousarray(np.tile(freqs_sin, (1, 2)))
    ii = np.arange(128)[:, None, None]
    tt = np.arange(4)[None, :, None]
    jj = np.arange(512)[None, None, :]
    masks = (jj >= ii + 128 * tt).astype(np.float32)
    ones = np.ones((128, 1), np.float32)
    eye = np.eye(128, dtype=np.float32)
    pfT = np.ascontiguousarray(prefix[0].T)
    g = np.tanh(prefix_gate)

    xTs = [np.ascontiguousarray(x[b].T) for b in range(B)]
    in_maps = []
    for c in range(NCORES):
        b, gi = divmod(c, CPB)
        cols = slice(gi * COLS, (gi + 1) * COLS)
        wqkv = np.ascontiguousarray(
            np.concatenate([wq[:, cols], wk[:, cols], wv[:, cols]], axis=1)
        )
        in_maps.append(
            dict(
                xT=xTs[b],
                wqkv=wqkv,
                wo=np.ascontiguousarray(wo[cols, :]),
                pfT=pfT,
                cosS=cosS,
                sinS=sinS,
                masks=masks,
                ones=ones,
                eye=eye,
                g=np.ascontiguousarray(g[None, gi * HPC : (gi + 1) * HPC]),
            )
        )
    return in_maps


def _run(inputs, trace=False, mm_fp32r=True):
    from concourse.bass_utils import run_bass_kernel_spmd

    key = ("nc", mm_fp32r)
    if key not in _CACHE:
        _CACHE[key] = _build(mm_fp32r)
    nc = _CACHE[key]
    in_maps = _host_inputs(
        inputs["x"],
        inputs["freqs_cos"],
        inputs["freqs_sin"],
        inputs["prefix"],
        inputs["prefix_gate"],
        inputs["wq"],
        inputs["wk"],
        inputs["wv"],
        inputs["wo"],
    )
    res = run_bass_kernel_spmd(nc, in_maps, list(range(NCORES)), trace=trace)
    parts = [res.results[c]["out"] for c in range(NCORES)]
    out = np.stack(
        [
            parts[0] + parts[1] + parts[2] + parts[3],
            parts[4] + parts[5] + parts[6] + parts[7],
        ],
        axis=0,
    ).astype(np.float32)
    return out, res


def kernel(**inputs) -> np.ndarray:
    out, _ = _run(inputs, trace=False)
    return out



# revision 11
# speedup vs baseline: 1.3674x; 1.3674x over previous
"""Trainium2 Bass kernel for nn_Attention_46901042872408 (v2, bf16).

Dense MHA transformer block with RoPE + prefix-tuning branch:
  q/k/v = x @ wq/wk/wv; rope(q), rope(k); causal attention;
  prefix branch: non-causal attention of q against (prefix @ wk/wv),
  gated by tanh(prefix_gate) per head; out = (attn + gate*prefix_attn) @ wo.

Sharding: 8 cores = data-parallel over batch (2) x tensor-parallel over
heads (4 groups of 8 heads). Each core computes a partial [2048, 4096]
output (its heads' contribution through its wo row-slice); host sums the
4 partials per batch. All matmul data is bf16 (fp32 PSUM accumulation);
rel-err budget is 2e-2 so bf16 (~0.4%/tensor) is comfortably inside.

Key structural choices vs the fp32r v1 baseline:
  - q/k are computed DIRECTLY in [head_dim, token] layout (weights
    stationary, x^T moving) so no PE-transposes and no DRAM spill of q/k.
  - RoPE without cross-partition ops: the qk weight columns are packed
    host-side into per-head block pairs A(h)=[wq_even|wk_even],
    B(h)=[wq_odd|wk_odd] (64+64 cols). Then rot_even = A*cos - B*sin and
    rot_odd = A*sin + B*cos are plain elementwise DVE ops. A cheap pair of
    permutation matmuls re-stacks (even;odd) halves into per-head
    [128, tok] q^T / k^T resident tiles (head-dim order is the
    de-interleaved permutation, consistently applied to q, k and
    prefix-k, so scores are unchanged).
  - v is computed straight [tok, col] (x stationary, wv moving), spilled
    to DRAM per chunk, reloaded once at phase 2 (SBUF capacity).
  - Phase 2 per (q-block, head): scores^T tiles [k_tok, q_tok] via one
    matmul per 128-k-tile, exp on ACT (bf16 out), causal mask multiply on
    the 4 diagonal tiles, PV + ones-row denominator accumulated in PSUM;
    prefix branch with 30 k-rows; combine normalizes PV in-PSUM with a
    broadcast reciprocal and writes attT [hd, tok] resident bf16.
  - Phase 3: out = attT.T @ wo streaming wo in 512-col slices; bf16 out,
    host upcasts and reduces the 4 head-group partials per batch.
  - prefix @ wk / prefix @ wv (0.06% of FLOPs) are computed host-side.
"""

import sys

sys.path.insert(0, "/opt/trn_rl_repo")

import numpy as np
import ml_dtypes

B, S, D = 2, 2048, 4096
H, HD = 32, 128
PFX = 30
NCORES = 8
CPB = 4  # cores per batch (head-parallel groups)
HPC = 8  # heads per core
COLS = HPC * HD  # 1024 qkv columns per core
NKT = D // 128  # 32 contraction tiles
NCH = 4  # token chunks in phase 1
CT = S // NCH  # 512 tokens per chunk
SCALE = 1.0 / float(np.sqrt(HD))

# column offsets inside the packed bf16 `misc` tensor [128, M_MISC]
O_COS = 0
O_SIN = O_COS + S
O_PERM = O_SIN + S  # 4 x 128 permutation matrices
O_MASK = O_PERM + 512  # 4 x 512 causal masks for the diagonal 512-block
O_ONES = O_MASK + 2048  # [128, 1] ones column
O_PKT = O_ONES + 1  # 8 x 30 prefix-k^T (de-interleaved head dim)
O_PV = O_PKT + HPC * PFX  # 8 x 128 prefix-v (rows 0..29)
M_MISC = 8192

_CACHE = {}


def _build(mm_fp32r=True):
    import os
    from contextlib import ExitStack

    import concourse.tile as tile
    from concourse import bacc, mybir

    f32 = mybir.dt.float32
    bf16 = mybir.dt.bfloat16
    AF = mybir.ActivationFunctionType
    OP = mybir.AluOpType

    phases = os.environ.get("KPHASES", "123")
    bars = os.environ.get("KBARS", "")

    def knob(name, default):
        return int(os.environ.get(name, default))

    nc = bacc.Bacc("TRN2", target_bir_lowering=False, debug=False, num_devices=NCORES)

    xT = nc.dram_tensor("xT", [D, S], bf16, kind="ExternalInput")
    wab = nc.dram_tensor("wab", [D, 2 * COLS], bf16, kind="ExternalInput")
    wv_d = nc.dram_tensor("wv", [D, COLS], bf16, kind="ExternalInput")
    wo_d = nc.dram_tensor("wo", [COLS, D], bf16, kind="ExternalInput")
    misc_d = nc.dram_tensor("misc", [128, M_MISC], bf16, kind="ExternalInput")
    g_d = nc.dram_tensor("g", [1, HPC], f32, kind="ExternalInput")
    out_d = nc.dram_tensor("out", [S, D], bf16, kind="ExternalOutput")

    with tile.TileContext(nc) as tc:
        with ExitStack() as top:
            dram = top.enter_context(tc.tile_pool(name="dram", bufs=1, space="DRAM"))
            v_sp = dram.tile([S, COLS], bf16)

            pres = top.enter_context(tc.tile_pool(name="res", bufs=1))
            misc = pres.tile([128, M_MISC], bf16)
            nc.sync.dma_start(misc[:, :O_PERM], misc_d[:, :O_PERM])
            g_sb = pres.tile([1, HPC], f32)
            qT = pres.tile([128, HPC, S], bf16)
            kT = pres.tile([128, HPC, S], bf16)

            # ---------------- Phase 1: projections + rope ----------------
            with ExitStack() as ph1:
              if "1" in phases:
                px = ph1.enter_context(tc.tile_pool(name="px", bufs=knob("B_px", 2)))
                pwA = ph1.enter_context(tc.tile_pool(name="pwA", bufs=knob("B_pwA", 2)))
                pwV = ph1.enter_context(tc.tile_pool(name="pwV", bufs=knob("B_pwV", 2)))
                pstg = ph1.enter_context(tc.tile_pool(name="pstg", bufs=knob("B_stg", 1)))
                ptmp = ph1.enter_context(tc.tile_pool(name="ptmp", bufs=1))
                pvs = ph1.enter_context(tc.tile_pool(name="pvs", bufs=1))
                ps_ab = ph1.enter_context(
                    tc.tile_pool(name="ps_ab", bufs=knob("B_psab", 2), space="PSUM")
                )
                ps_qk = ph1.enter_context(
                    tc.tile_pool(name="ps_qk", bufs=1, space="PSUM")
                )
                ps_v = ph1.enter_context(tc.tile_pool(name="ps_v", bufs=2, space="PSUM"))

                for c in range(NCH):
                    t0 = c * CT
                    x_sb = px.tile([128, NKT, CT], bf16, tag="x")
                    nc.sync.dma_start(
                        x_sb[:, : NKT // 2, :],
                        xT[: D // 2, t0 : t0 + CT].rearrange(
                            "(ko p) n -> p ko n", p=128
                        ),
                    )
                    nc.sync.dma_start(
                        x_sb[:, NKT // 2 :, :],
                        xT[D // 2 :, t0 : t0 + CT].rearrange(
                            "(ko p) n -> p ko n", p=128
                        ),
                    )
                    cc = misc[:, O_COS + t0 : O_COS + t0 + CT]
                    ss = misc[:, O_SIN + t0 : O_SIN + t0 + CT]
                    for h in range(HPC):
                        wA = pwA.tile([128, NKT, 128], bf16, tag="wA")
                        nc.sync.dma_start(
                            wA[:],
                            wab[:, (2 * h) * 128 : (2 * h + 1) * 128].rearrange(
                                "(ko p) c -> p ko c", p=128
                            ),
                        )
                        wB = pwA.tile([128, NKT, 128], bf16, tag="wB")
                        nc.sync.dma_start(
                            wB[:],
                            wab[:, (2 * h + 1) * 128 : (2 * h + 2) * 128].rearrange(
                                "(ko p) c -> p ko c", p=128
                            ),
                        )
                        if c == 0 and h == 0:
                            nc.sync.dma_start(
                                misc[:, O_PERM:], misc_d[:, O_PERM:]
                            )
                            nc.sync.dma_start(g_sb[:], g_d[:])
                        psA = ps_ab.tile([128, CT], f32, tag="A")
                        psB = ps_ab.tile([128, CT], f32, tag="B")
                        for ki in range(NKT):
                            nc.tensor.matmul(
                                psA[:],
                                lhsT=wA[:, ki, :],
                                rhs=x_sb[:, ki, :],
                                start=(ki == 0),
                                stop=(ki == NKT - 1),
                            )
                        for ki in range(NKT):
                            nc.tensor.matmul(
                                psB[:],
                                lhsT=wB[:, ki, :],
                                rhs=x_sb[:, ki, :],
                                start=(ki == 0),
                                stop=(ki == NKT - 1),
                            )
                        m1 = ptmp.tile([128, CT], f32, tag="m1")
                        m2 = ptmp.tile([128, CT], f32, tag="m2")
                        stgA = pstg.tile([128, CT], bf16, tag="sA")
                        stgB = pstg.tile([128, CT], bf16, tag="sB")
                        nc.vector.tensor_tensor(m1[:], psA[:], cc, OP.mult)
                        nc.vector.tensor_tensor(m2[:], psB[:], ss, OP.mult)
                        nc.vector.tensor_tensor(stgA[:], m1[:], m2[:], OP.subtract)
                        m3 = ptmp.tile([128, CT], f32, tag="m1")
                        m4 = ptmp.tile([128, CT], f32, tag="m2")
                        nc.vector.tensor_tensor(m3[:], psA[:], ss, OP.mult)
                        nc.vector.tensor_tensor(m4[:], psB[:], cc, OP.mult)
                        nc.vector.tensor_tensor(stgB[:], m3[:], m4[:], OP.add)
                        psq = ps_qk.tile([128, CT], f32, tag="q")
                        nc.tensor.matmul(
                            psq[:],
                            lhsT=misc[:, O_PERM : O_PERM + 128],
                            rhs=stgA[:],
                            start=True,
                            stop=False,
                        )
                        nc.tensor.matmul(
                            psq[:],
                            lhsT=misc[:, O_PERM + 128 : O_PERM + 256],
                            rhs=stgB[:],
                            start=False,
                            stop=True,
                        )
                        nc.scalar.activation(qT[:, h, t0 : t0 + CT], psq[:], AF.Copy)
                        psk = ps_qk.tile([128, CT], f32, tag="k")
                        nc.tensor.matmul(
                            psk[:],
                            lhsT=misc[:, O_PERM + 256 : O_PERM + 384],
                            rhs=stgA[:],
                            start=True,
                            stop=False,
                        )
                        nc.tensor.matmul(
                            psk[:],
                            lhsT=misc[:, O_PERM + 384 : O_PERM + 512],
                            rhs=stgB[:],
                            start=False,
                            stop=True,
                        )
                        nc.scalar.activation(kT[:, h, t0 : t0 + CT], psk[:], AF.Copy)

                    # v for this chunk: straight [tok, col], spill to DRAM
                    vst = pvs.tile([128, CT // 128, COLS], bf16, tag="vst")
                    for vb in range(COLS // 128):
                        wV = pwV.tile([128, NKT, 128], bf16, tag="wV")
                        nc.sync.dma_start(
                            wV[:],
                            wv_d[:, vb * 128 : (vb + 1) * 128].rearrange(
                                "(ko p) c -> p ko c", p=128
                            ),
                        )
                        for tt in range(CT // 128):
                            psv = ps_v.tile([128, 128], f32, tag="v")
                            for ki in range(NKT):
                                nc.tensor.matmul(
                                    psv[:],
                                    lhsT=x_sb[:, ki, tt * 128 : (tt + 1) * 128],
                                    rhs=wV[:, ki, :],
                                    start=(ki == 0),
                                    stop=(ki == NKT - 1),
                                )
                            nc.scalar.activation(
                                vst[:, tt, vb * 128 : (vb + 1) * 128], psv[:], AF.Copy
                            )
                    nc.sync.dma_start(
                        v_sp[t0 : t0 + CT, :].rearrange("(tt p) c -> p tt c", p=128),
                        vst[:],
                    )

            if "12" in bars:
                tc.strict_bb_all_engine_barrier()

            # ---------------- Phases 2+3 ----------------
            with ExitStack() as ph23:
                patt = ph23.enter_context(tc.tile_pool(name="patt", bufs=1))
                attT = patt.tile([128, HPC, S], bf16)
                pwo = ph23.enter_context(
                    tc.tile_pool(name="pwo", bufs=knob("B_pwo", 2))
                )

                with ExitStack() as ph2:
                  if "2" in phases:
                    pv2 = ph2.enter_context(tc.tile_pool(name="pv2", bufs=1))
                    v_sb = pv2.tile([128, S // 128, COLS], bf16)
                    for c in range(NCH):
                        nc.sync.dma_start(
                            v_sb[:, c * (CT // 128) : (c + 1) * (CT // 128), :],
                            v_sp[c * CT : (c + 1) * CT, :].rearrange(
                                "(kb p) c -> p kb c", p=128
                            ),
                        )
                    pE = ph2.enter_context(tc.tile_pool(name="pE", bufs=knob("B_pE", 5)))
                    pEP = ph2.enter_context(tc.tile_pool(name="pEP", bufs=2))
                    pc = ph2.enter_context(tc.tile_pool(name="pc", bufs=knob("B_pc", 2)))
                    ps_s = ph2.enter_context(
                        tc.tile_pool(name="ps_s", bufs=knob("B_pss", 3), space="PSUM")
                    )
                    ps_pv = ph2.enter_context(
                        tc.tile_pool(name="ps_pv", bufs=2, space="PSUM")
                    )
                    ps_sd = ph2.enter_context(
                        tc.tile_pool(name="ps_sd", bufs=2, space="PSUM")
                    )
                    ps_pp = ph2.enter_context(
                        tc.tile_pool(name="ps_pp", bufs=1, space="PSUM")
                    )

                    LA = knob("KLA", 2)
                    fronts = []
                    backs = []
                    for qb in range(4):
                        for h in range(HPC):
                            q0 = qb * 512
                            qap = qT[:, h, q0 : q0 + 512]
                            nkb = 4 * qb + 4
                            st = {}

                            def pfx_front(qb=qb, h=h, qap=qap, st=st):
                                spdd = ps_sd.tile([65, 512], f32, tag="spdd")
                                st["sp"] = spdd[0:PFX, :]
                                st["den"] = spdd[32:33, :]
                                st["denP"] = spdd[64:65, :]
                                st["pv"] = ps_pv.tile([128, 512], f32, tag="pv", name="pv")
                                nc.tensor.matmul(
                                    st["sp"],
                                    lhsT=misc[:, O_PKT + h * PFX : O_PKT + (h + 1) * PFX],
                                    rhs=qap,
                                    start=True,
                                    stop=True,
                                )
                                EP = pEP.tile([PFX, 512], bf16, tag="EP")
                                nc.scalar.activation(EP[:], st["sp"], AF.Exp, scale=SCALE)
                                st["EP"] = EP

                            def s_front(kb, qb=qb, h=h, qap=qap, st=st):
                                s_ps = ps_s.tile([128, 512], f32, tag="s")
                                nc.tensor.matmul(
                                    s_ps[:],
                                    lhsT=kT[:, h, kb * 128 : (kb + 1) * 128],
                                    rhs=qap,
                                    start=True,
                                    stop=True,
                                )
                                E = pE.tile([128, 512], bf16, tag="E")
                                nc.scalar.activation(E[:], s_ps[:], AF.Exp, scale=SCALE)
                                t = kb - 4 * qb
                                if t >= 0:
                                    nc.vector.tensor_tensor(
                                        E[:],
                                        E[:],
                                        misc[:, O_MASK + t * 512 : O_MASK + (t + 1) * 512],
                                        OP.mult,
                                    )
                                st.setdefault("E", {})[kb] = E

                            def s_back(kb, qb=qb, h=h, st=st, nkb=nkb):
                                E = st["E"].pop(kb)
                                nc.tensor.matmul(
                                    st["pv"],
                                    lhsT=v_sb[:, kb, h * 128 : (h + 1) * 128],
                                    rhs=E[:],
                                    start=(kb == 0),
                                    stop=(kb == nkb - 1),
                                )
                                nc.tensor.matmul(
                                    st["den"],
                                    lhsT=misc[:, O_ONES : O_ONES + 1],
                                    rhs=E[:],
                                    start=(kb == 0),
                                    stop=(kb == nkb - 1),
                                )

                            def pfx_back(qb=qb, h=h, q0=q0, st=st):
                                pvP_ps = ps_pp.tile([128, 512], f32, tag="pvP")
                                nc.tensor.matmul(
                                    pvP_ps[:],
                                    lhsT=misc[0:PFX, O_PV + h * 128 : O_PV + (h + 1) * 128],
                                    rhs=st["EP"][:],
                                    start=True,
                                    stop=True,
                                )
                                nc.tensor.matmul(
                                    st["denP"],
                                    lhsT=misc[0:PFX, O_ONES : O_ONES + 1],
                                    rhs=st["EP"][:],
                                    start=True,
                                    stop=True,
                                )
                                r12 = pc.tile([1, 1024], f32, tag="r12")
                                nc.vector.reciprocal(r12[:, 0:512], st["den"])
                                nc.vector.reciprocal(r12[:, 512:1024], st["denP"])
                                nc.vector.tensor_scalar_mul(
                                    r12[:, 512:1024], r12[:, 512:1024], g_sb[0:1, h : h + 1]
                                )
                                rb = pc.tile([128, 1024], f32, tag="rb")
                                nc.gpsimd.partition_broadcast(rb[:], r12[:])
                                nc.vector.tensor_tensor(
                                    st["pv"], st["pv"], rb[:, 0:512], OP.mult
                                )
                                t2 = pc.tile([128, 512], bf16, tag="t2")
                                nc.vector.tensor_tensor(
                                    t2[:], pvP_ps[:], rb[:, 512:1024], OP.mult
                                )
                                nc.vector.tensor_tensor(
                                    attT[:, h, q0 : q0 + 512], st["pv"], t2[:], OP.add
                                )

                            fronts.append(pfx_front)
                            backs.extend(
                                (lambda kb=kb, f=s_back: f(kb)) for kb in range(nkb)
                            )
                            fronts.extend(
                                (lambda kb=kb, f=s_front: f(kb)) for kb in range(nkb)
                            )
                            backs.append(pfx_back)

                    assert len(fronts) == len(backs)
                    for i in range(min(LA, len(fronts))):
                        fronts[i]()
                    for i in range(len(backs)):
                        if i + LA < len(fronts):
                            fronts[i + LA]()
                        backs[i]()

                if "23" in bars:
                    tc.strict_bb_all_engine_barrier()

                # ---------------- Phase 3: output projection ----------------
                with ExitStack() as ph3:
                  if "3" in phases:
                    post = ph3.enter_context(tc.tile_pool(name="post", bufs=knob("B_post", 2)))
                    ps3 = ph3.enter_context(
                        tc.tile_pool(name="ps3", bufs=knob("B_ps3", 4), space="PSUM")
                    )
                    for nb in range(D // 512):
                        wo_sb = pwo.tile([128, HPC, 512], bf16, tag="wo")
                        nc.sync.dma_start(
                            wo_sb[:],
                            wo_d[:, nb * 512 : (nb + 1) * 512].rearrange(
                                "(ko p) d -> p ko d", p=128
                            ),
                        )
                        ost = post.tile([128, S // 128, 512], bf16, tag="o")
                        for mt in range(S // 128):
                            ps = ps3.tile([128, 512], f32, tag="mm3")
                            for kc in range(HPC):
                                nc.tensor.matmul(
                                    ps[:],
                                    lhsT=attT[:, kc, mt * 128 : (mt + 1) * 128],
                                    rhs=wo_sb[:, kc, :],
                                    start=(kc == 0),
                                    stop=(kc == HPC - 1),
                                )
                            nc.scalar.activation(ost[:, mt, :], ps[:], AF.Copy)
                        nc.sync.dma_start(
                            out_d[:, nb * 512 : (nb + 1) * 512].rearrange(
                                "(mt p) c -> p mt c", p=128
                            ),
                            ost[:],
                        )

    nc.compile()
    return nc


def _host_inputs(x, freqs_cos, freqs_sin, prefix, prefix_gate, wq, wk, wv, wo):
    bf = ml_dtypes.bfloat16
    x = np.asarray(x, np.float32)
    freqs_cos = np.asarray(freqs_cos, np.float32)
    freqs_sin = np.asarray(freqs_sin, np.float32)
    prefix = np.asarray(prefix, np.float32)
    prefix_gate = np.asarray(prefix_gate, np.float32)
    wq = np.asarray(wq, np.float32)
    wk = np.asarray(wk, np.float32)
    wv = np.asarray(wv, np.float32)
    wo = np.asarray(wo, np.float32)

    deint = np.r_[np.arange(0, HD, 2), np.arange(1, HD, 2)]

    fcT = freqs_cos.T  # [64, S]
    fsT = freqs_sin.T
    cos128 = np.concatenate([fcT, fcT], 0)  # [128, S]
    sin128 = np.concatenate([fsT, fsT], 0)
    perm4 = np.zeros((128, 4, 128), np.float32)
    i = np.arange(64)
    perm4[i, 0, i] = 1.0  # q even half <- stgA top
    perm4[i, 1, 64 + i] = 1.0  # q odd half <- stgB top
    perm4[64 + i, 2, i] = 1.0  # k even half <- stgA bottom
    perm4[64 + i, 3, 64 + i] = 1.0  # k odd half <- stgB bottom
    ii = np.arange(128)[:, None, None]
    tt_ = np.arange(4)[None, :, None]
    jj = np.arange(512)[None, None, :]
    masks = (jj >= ii + 128 * tt_).astype(np.float32)

    misc_tpl = np.zeros((128, M_MISC), np.float32)
    misc_tpl[:, O_COS : O_COS + S] = cos128
    misc_tpl[:, O_SIN : O_SIN + S] = sin128
    misc_tpl[:, O_PERM : O_PERM + 512] = perm4.reshape(128, 512)
    misc_tpl[:, O_MASK : O_MASK + 2048] = masks.reshape(128, 2048)
    misc_tpl[:, O_ONES] = 1.0

    xTs = [np.ascontiguousarray(x[b].T).astype(bf) for b in range(B)]
    pk_full = prefix[0] @ wk  # [30, 4096]
    pv_full = prefix[0] @ wv
    g_full = np.tanh(prefix_gate)

    in_maps = []
    for c in range(NCORES):
        b, gi = divmod(c, CPB)
        cs = slice(gi * COLS, (gi + 1) * COLS)
        wq_g = wq[:, cs].reshape(D, HPC, HD)
        wk_g = wk[:, cs].reshape(D, HPC, HD)
        wab = np.empty((D, HPC, 2, 128), np.float32)
        wab[:, :, 0, :64] = wq_g[:, :, 0::2]
        wab[:, :, 0, 64:] = wk_g[:, :, 0::2]
        wab[:, :, 1, :64] = wq_g[:, :, 1::2]
        wab[:, :, 1, 64:] = wk_g[:, :, 1::2]

        misc = misc_tpl.copy()
        pk_g = pk_full[:, cs].reshape(PFX, HPC, HD)
        pkT = np.transpose(pk_g[:, :, deint], (2, 1, 0))  # [128, 8, 30]
        misc[:, O_PKT : O_PKT + HPC * PFX] = pkT.reshape(128, HPC * PFX)
        misc[0:PFX, O_PV : O_PV + COLS] = pv_full[:, cs]

        in_maps.append(
            dict(
                xT=xTs[b],
                wab=np.ascontiguousarray(wab.reshape(D, 2 * COLS)).astype(bf),
                wv=np.ascontiguousarray(wv[:, cs]).astype(bf),
                wo=np.ascontiguousarray(wo[cs, :]).astype(bf),
                misc=misc.astype(bf),
                g=np.ascontiguousarray(
                    g_full[None, gi * HPC : (gi + 1) * HPC]
                ).astype(np.float32),
            )
        )
    return in_maps


def _run(inputs, trace=False, mm_fp32r=True):
    from concourse.bass_utils import run_bass_kernel_spmd

    key = ("nc", mm_fp32r)
    if key not in _CACHE:
        _CACHE[key] = _build(mm_fp32r)
    nc = _CACHE[key]
    in_maps = _host_inputs(
        inputs["x"],
        inputs["freqs_cos"],
        inputs["freqs_sin"],
        inputs["prefix"],
        inputs["prefix_gate"],
        inputs["wq"],
        inputs["wk"],
        inputs["wv"],
        inputs["wo"],
    )
    res = run_bass_kernel_spmd(nc, in_maps, list(range(NCORES)), trace=trace)
    parts = [np.asarray(res.results[c]["out"], np.float32) for c in range(NCORES)]
    out = np.stack(
        [
            parts[0] + parts[1] + parts[2] + parts[3],
            parts[4] + parts[5] + parts[6] + parts[7],
        ],
        axis=0,
    )
    return out.astype(np.float32), res


def kernel(**inputs) -> np.ndarray:
    out, _ = _run(inputs, trace=False)
    return out
